# revision 16
# baseline (speedup 1.0000x reference)
"""Trainium2 Bass kernel for nn_GAT_87617332838818.

Mathematical collapse: the reference GAT's softmax weights sum to 1 within
each destination segment and the aggregated message ``hp[dst]`` is constant
within the segment, so message passing is the identity and the network is a
per-node 3-layer MLP:

    logits = W2r @ elu(W1r @ elu(W0r @ x^T))      (per node column)

with W0r = W0.reshape(96,128), W1r = W1.reshape(96,96), W2r = W2.reshape(40,96).

Device strategy (8 NeuronCores, node-sharded 6250 cols each), v2:
  - exact ELU with zero bias bookkeeping:  elu(p) = max(p,0) + (min(exp(p),1) - 1)
    r-pass:  DVE/ACT tensor_scalar max(p,0)        (PSUM read, 1x)
    e-pass:  ACT exp(p)                            (PSUM read, 1x)
    t'-pass: DVE (e min 1) add -1                  (SBUF fp16, 4x mode)
    both halves feed two accumulating matmuls (linearity), so no +1
    inflation ever exists and no per-layer bias corrections are needed.
  - supergroups of 1024 columns: ps0/ps1 PSUM tiles span 2 banks, so each
    drain pass covers 1024 cols in ONE instruction (halves instr count and
    the ~130ns/instr semaphore tax).  L2's [104,512] pair-packed output is
    written into ps1's bank 0 after the L1 drains release it, keeping the
    whole pipeline in exactly 8 PSUM banks (ps0 2x2 + ps1 2x2).
  - output drained as [104,512] (rows 0:40 = even 512-col group,
    64:104 = odd) then DMA'd as two clean [40,512] row-slices into a
    contiguous yT[40,6250] — no padded columns in the output DMA.
  - engine balance: exp is ACT-only; r/out drains are split ACT/DVE by
    static assignment sets tuned from traces.
  - warmup matmuls flip the PE p-state to 2.4 GHz during the DMA head.
"""

import os
import sys

import numpy as np

for _p in ("/root/.axon_site/_ro/trn_rl_repo", "/opt/trn_rl_repo"):
    if os.path.isdir(_p) and _p not in sys.path:
        sys.path.append(_p)

import concourse.bass as bass
import concourse.tile as tile
from concourse import bacc, mybir
from concourse.bass_utils import run_bass_kernel_spmd

N_CORES = 8
N_PER = 6250            # 50000 / 8
D_IN = 128
D_HID = 96
D_OUT = 40
BANK = 512              # matmul free-dim limit (1 PSUM bank of f32)
SGW = 1024              # supergroup width (2 PSUM banks)

F16 = mybir.dt.float16
F32 = mybir.dt.float32

Act = mybir.ActivationFunctionType
Alu = mybir.AluOpType

# supergroups: (start_col, width)
SGS = []
_c = 0
while _c < N_PER:
    SGS.append((_c, min(SGW, N_PER - _c)))
    _c += SGW
NSG = len(SGS)          # 7: six 1024-wide + one 106-wide tail

N_WARMUP_MM = int(os.environ.get("GAT_WARMUP", "10"))
N_INLINE_JUNK = int(os.environ.get("GAT_INLINE_JUNK", "2"))
INLINE_JUNK_STEPS = 3   # steps that get gap-filling junk matmuls
# which r-drains run on ACT instead of DVE, by (sg, layer)
R_ON_ACT = tuple((s, 1) for s in range(NSG))
# which out-drains run on ACT instead of DVE
OUT_ON_ACT = ()

# input DMA batches, by 512-col chunk index (chunks 0..12, chunk 12 = tail).
# batch 0 additionally carries W0^T. All issued up front on the sync ring.
X_BATCHES = [1, 4, 4, 4]
_chunk_batch = {}
_c0 = 0
for _bi, _bn in enumerate(X_BATCHES):
    for _ch in range(_c0, _c0 + _bn):
        _chunk_batch[_ch] = _bi
    _c0 += _bn
N_CHUNKS = (N_PER + BANK - 1) // BANK
assert _c0 >= N_CHUNKS


def _splits(w):
    out = []
    j = 0
    while j < w:
        out.append((j, min(j + BANK, w)))
        j += BANK
    return out


def _build_program() -> bass.Bass:
    nc = bacc.Bacc(None, target_bir_lowering=False, debug=False)

    # xw packs [w0t | xT]: cols 0..95 = W0^T fp16, cols 96.. = x^T shard
    xw = nc.declare_dram_parameter("xw", [D_IN, D_HID + N_PER], F16,
                                   isOutput=False)
    # wb packs [w1t | w2t] fp16
    wb = nc.declare_dram_parameter("wb", [D_HID, D_HID + D_OUT], F16,
                                   isOutput=False)
    yT = nc.declare_dram_parameter("yT", [D_OUT, N_PER], F16, isOutput=True)

    st = {}
    shared = {}

    with tile.TileContext(nc) as tc:
        with (
            tc.tile_pool(name="consts", bufs=1) as consts,
            tc.tile_pool(name="xin", bufs=1) as xpool,
            tc.tile_pool(name="sb", bufs=2) as sb,
            tc.tile_pool(name="ps0", bufs=1, space="PSUM") as ps0,
            tc.tile_pool(name="ps1", bufs=2, space="PSUM") as ps1,
            tc.tile_pool(name="ps2", bufs=2, space="PSUM") as ps2,
        ):
            # --- PE warm-up on junk SBUF during the DMA-bound head.
            # memset via DVE: its program loads early and it is otherwise
            # idle until ~8.5us (a GpSimd memset would gate the warm-up
            # behind the Pool program load at ~6us).
            junk_w = consts.tile([D_IN, D_OUT], F16, tag="junkw")
            junk_x = consts.tile([D_IN, BANK], F16, tag="junkx")
            nc.vector.memset(junk_w[:], 0.0)
            nc.vector.memset(junk_x[:], 0.0)
            # tiny dummy exp: forces the walrus-inserted ACT_TABLE_LOAD
            # (~1.3us) to run during the DMA head instead of gating the
            # first real exp pass.
            junk_e = consts.tile([1, 4], F16, tag="junke")
            nc.scalar.activation(junk_e[:], junk_w[:1, :4], Act.Exp)
            warm = ps2.tile([104, BANK], F32, tag="p2")
            for _ in range(N_WARMUP_MM):
                nc.tensor.matmul(warm[:D_OUT, :BANK], junk_w[:], junk_x[:],
                                 start=True, stop=True)

            wb_sb = consts.tile([D_HID, D_HID + D_OUT], F16, tag="wb")
            w1_sb = wb_sb[:, :D_HID]
            w2_sb = wb_sb[:, D_HID:D_HID + D_OUT]

            def rdrain(out_ap, psum_ap, on_act):
                """out = max(psum, 0), PSUM -> SBUF fp16."""
                if on_act:
                    nc.scalar.activation(out_ap, psum_ap, Act.Relu)
                else:
                    nc.vector.tensor_scalar_max(out_ap, psum_ap, 0.0)

            # --- input DMA batches: one tile per batch, all issued up
            # front on the sync ring so outputs can never head-block them.
            batch_tiles = {}

            def load_all():
                ch = 0
                for bi, bn in enumerate(X_BATCHES):
                    if ch >= N_CHUNKS:
                        break
                    lo = ch * BANK + (0 if bi else -D_HID)
                    hi = min((ch + bn) * BANK, N_PER)
                    xt = xpool.tile([D_IN, hi - lo], F16, tag=f"xt{bi}")
                    nc.sync.dma_start(xt[:], xw[:, D_HID + lo:D_HID + hi])
                    batch_tiles[bi] = {"xt": xt, "base": lo}
                    if bi == 0:
                        shared["w0"] = xt[:, 0:D_HID]
                        nc.sync.dma_start(wb_sb[:], wb[:])
                    ch += bn

            def xsrc(chunk):
                """(tile, col offset) holding x columns of this 512-chunk."""
                b = batch_tiles[_chunk_batch[chunk]]
                return b["xt"], chunk * BANK - b["base"]

            def stage0_mm(s):
                c0, w = SGS[s]
                d = st.setdefault(s, {})
                p0 = ps0.tile([D_HID, SGW], F32, tag="p0")
                if s < INLINE_JUNK_STEPS:
                    # gap-filling junk matmuls into the bank the real L0
                    # matmul is about to overwrite: keeps the PE busy-streak
                    # alive so the p-state governor ramps to 2.4 GHz without
                    # delaying real work (both wait on the same drains).
                    for _ in range(N_INLINE_JUNK):
                        nc.tensor.matmul(p0[:D_OUT, :BANK], junk_w[:],
                                         junk_x[:], start=True, stop=True,
                                         skip_group_check=True)
                for j0, j1 in _splits(w):
                    xt, xo = xsrc((c0 + j0) // BANK)
                    nc.tensor.matmul(p0[:, j0:j1], shared["w0"],
                                     xt[:, xo:xo + (j1 - j0)],
                                     start=True, stop=True)
                d["p0"] = p0

            def stage0_dr(s):
                w = SGS[s][1]
                d = st[s]
                p0 = d.pop("p0")
                e0 = sb.tile([D_HID, SGW], F16, tag="e0")
                r0 = sb.tile([D_HID, SGW], F16, tag="r0")
                t0 = sb.tile([D_HID, SGW], F16, tag="t0")
                nc.scalar.activation(e0[:, :w], p0[:, :w], Act.Exp)
                rdrain(r0[:, :w], p0[:, :w], (s, 0) in R_ON_ACT)
                nc.vector.tensor_scalar(t0[:, :w], e0[:, :w], 1.0, -1.0,
                                        Alu.min, Alu.add)
                d["r0"], d["t0"] = r0, t0

            def stage1_mm(s):
                w = SGS[s][1]
                d = st[s]
                p1 = ps1.tile([D_HID, SGW], F32, tag="p1")
                for j0, j1 in _splits(w):
                    nc.tensor.matmul(p1[:, j0:j1], w1_sb,
                                     d["r0"][:, j0:j1], start=True, stop=False)
                    nc.tensor.matmul(p1[:, j0:j1], w1_sb,
                                     d["t0"][:, j0:j1], start=False, stop=True)
                d["p1"] = p1

            def stage1_dr(s):
                w = SGS[s][1]
                d = st[s]
                p1 = d.pop("p1")
                e1 = sb.tile([D_HID, SGW], F16, tag="e1")
                r1 = sb.tile([D_HID, SGW], F16, tag="r1")
                t1 = sb.tile([D_HID, SGW], F16, tag="t1")
                nc.scalar.activation(e1[:, :w], p1[:, :w], Act.Exp)
                rdrain(r1[:, :w], p1[:, :w], (s, 1) in R_ON_ACT)
                nc.vector.tensor_scalar(t1[:, :w], e1[:, :w], 1.0, -1.0,
                                        Alu.min, Alu.add)
                d["r1"], d["t1"] = r1, t1

            def stage2_mm(s):
                w = SGS[s][1]
                d = st[s]
                p2 = ps2.tile([104, BANK], F32, tag="p2")
                w0_ = min(w, BANK)
                nc.tensor.matmul(p2[:D_OUT, :w0_], w2_sb, d["r1"][:, :w0_],
                                 start=True, stop=False)
                nc.tensor.matmul(p2[:D_OUT, :w0_], w2_sb, d["t1"][:, :w0_],
                                 start=False, stop=True)
                if w > BANK:
                    w1_ = w - BANK
                    nc.tensor.matmul(p2[64:64 + D_OUT, :w1_], w2_sb,
                                     d["r1"][:, BANK:w], start=True,
                                     stop=False)
                    nc.tensor.matmul(p2[64:64 + D_OUT, :w1_], w2_sb,
                                     d["t1"][:, BANK:w], start=False,
                                     stop=True)
                d["p2"] = p2

            def stage2_dr(s):
                c0, w = SGS[s]
                d = st.pop(s)
                p2 = d["p2"]
                nrows = 104 if w > BANK else D_OUT
                w0_ = min(w, BANK)
                o = sb.tile([104, BANK], F16, tag="o")
                if s in OUT_ON_ACT:
                    nc.scalar.activation(o[:nrows, :w0_], p2[:nrows, :w0_],
                                         Act.Identity)
                else:
                    nc.vector.tensor_copy(o[:nrows, :w0_], p2[:nrows, :w0_])
                nc.sync.dma_start(yT[:, c0:c0 + w0_], o[:D_OUT, :w0_])
                if w > BANK:
                    nc.sync.dma_start(yT[:, c0 + BANK:c0 + w],
                                      o[64:64 + D_OUT, :w - BANK])

            load_all()
            # Per-step emission order is engine-queue order.  PE runs
            # [L1mm(s-1), L2mm(s-2), L0mm(s)] so the ps0 bufs=1 WAR on
            # L0mm always sees its drains (early items of the previous
            # step) long done.  ACT runs [exp1(s-1), exp0(s)] so exp0's
            # same-step producer L0mm has finished by the time ACT gets
            # there.  DVE puts t0(s) last: its producer exp0(s) ends
            # mid-step on ACT.
            for step in range(NSG + 2):
                if 0 <= step - 1 < NSG:
                    stage1_mm(step - 1)
                if 0 <= step - 2 < NSG:
                    stage2_mm(step - 2)
                if step < NSG:
                    stage0_mm(step)
                if 0 <= step - 1 < NSG:
                    stage1_dr(step - 1)
                if 0 <= step - 2 < NSG:
                    stage2_dr(step - 2)
                if step < NSG:
                    stage0_dr(step)


    nc.compile()
    return nc


_prog_cache = []
last_result = None


def kernel(**inputs) -> np.ndarray:
    global last_result
    x = np.asarray(inputs["x"], np.float32)           # [50000, 128]
    W0 = np.asarray(inputs["W0"], np.float32).reshape(D_HID, D_IN)
    W1 = np.asarray(inputs["W1"], np.float32).reshape(D_HID, D_HID)
    W2 = np.asarray(inputs["W2"], np.float32).reshape(D_OUT, D_HID)

    n = x.shape[0]
    assert n == N_CORES * N_PER, f"unexpected node count {n}"

    xT16 = x.T.astype(np.float16)                            # [128, 50000]
    w0t = W0.T.astype(np.float16)                            # [128, 96]
    wbm = np.ascontiguousarray(np.concatenate(
        [W1.T.astype(np.float16), W2.T.astype(np.float16)], axis=1))

    if not _prog_cache:
        _prog_cache.append(_build_program())
    nc = _prog_cache[0]

    in_maps = []
    for i in range(N_CORES):
        xwi = np.ascontiguousarray(
            np.concatenate([w0t, xT16[:, i * N_PER:(i + 1) * N_PER]], axis=1))
        in_maps.append(dict(xw=xwi, wb=wbm))
    res = run_bass_kernel_spmd(nc, in_maps, list(range(N_CORES)))
    last_result = res
    out = np.empty((n, D_OUT), np.float32)
    for i in range(N_CORES):
        yt = np.asarray(res.results[i]["yT"], np.float32)  # [40, 6250]
        out[i * N_PER:(i + 1) * N_PER] = yt.T
    return out


if __name__ == "__main__":
    data = np.load("/tmp/gat_inputs.npz")
    y = kernel(**{k: data[k] for k in data.files})
    print("out", y.shape, y.dtype, "absmax", np.abs(y).max())


# revision 25
# speedup vs baseline: 1.2487x; 1.2487x over previous
"""Trainium2 Bass kernel for nn_GAT_87617332838818.

Mathematical collapse: the reference GAT aggregates ``alpha * hp[:, dst]``
over incoming edges per destination node.  Since the softmax weights alpha
sum to exactly 1 within each destination segment and the aggregated message
``hp[dst]`` is constant within the segment, the whole message-passing step
is the identity: ``out[n] = hp[n]``.  The network therefore reduces to a
per-node 3-layer MLP:

    logits = W2r @ elu(W1r @ elu(W0r @ x^T))        (per node column)

with W0r = W0.reshape(96,128), W1r = W1.reshape(96,96), W2r = W2.reshape(40,96)
(head-concat order matches the plain reshape).  Verified numerically against
the reference: rel fro err 4e-7 in f32; 4.5e-3 with this device pipeline.

Device strategy (8 NeuronCores, node-sharded 6250 rows each):
  - activations kept feature-on-partition: xT [128, n], h [96, n]
  - ELU via the split  elu(p') + 1 = max(p',0) + min(exp(p'),1)  with
    p' = p + nb (nb folds the "+1" inflation of the previous layer:
    nb = -W @ ones).  r = max(p+nb,0) and t = min(exp(p+nb),1) are fed
    through TWO accumulating matmuls (linearity), so the inflated h+1 is
    only ever formed in f32 PSUM — bf16-safe.
  - final layer bias cb2 = W2 @ ones subtracted in the output drain pass.
  - pipeline works on 512-column groups (one PSUM bank per matmul).  L2
    outputs of consecutive groups are packed vertically (partitions 0:40
    and 64:104 — PSUM base partitions must be 0/32/64) into one [104,512]
    PSUM tile so one drain pass and paired DMAs cover both groups.
  - PSUM drains split between DVE and ACT for engine balance (any pass
    reading f32 PSUM runs at 1x; only 16-bit SBUF passes get 2x/4x modes).
  - NOTE: engine passes whose PSUM AP spans two banks crashed the device
    (NRT_EXEC_UNIT_UNRECOVERABLE) — keep all PSUM APs within one bank.
  - 3-stage software-pipelined emission so each engine's in-order stream
    always has ready work (avoids head-of-line blocking across pairs).
  - dummy matmuls parked in the DMA-bound head flip the PE HAM clock
    gate to 2.4 GHz before the real matmuls start (measured 427->216 ns).
  - w0 rides in the first x DMA batch; w1/w2 and biases are packed into
    single DMAs to cut ~620 ns/issue sequencer serialization.
"""

import os
import sys

import numpy as np

for _p in ("/root/.axon_site/_ro/trn_rl_repo", "/opt/trn_rl_repo"):
    if os.path.isdir(_p) and _p not in sys.path:
        sys.path.append(_p)

import concourse.bass as bass
import concourse.tile as tile
from concourse import bacc, mybir
from concourse.bass_utils import run_bass_kernel_spmd

N_CORES = 8
N_PER = 6250            # 50000 / 8
D_IN = 128
D_HID = 96
D_OUT = 40
MM_N = 512              # matmul moving free-dim (1 PSUM bank)
FDP = 512               # group free-dim (1 PSUM bank)

F16 = mybir.dt.float16
BF16 = mybir.dt.bfloat16
F32 = mybir.dt.float32

Act = mybir.ActivationFunctionType
Alu = mybir.AluOpType

_pairs = [FDP] * (N_PER // FDP)
if N_PER % FDP:
    _pairs.append(N_PER % FDP)
P = len(_pairs)
_pstarts = [sum(_pairs[:i]) for i in range(P)]

# which L0/L1 relu drains go to ACT instead of DVE (by (pair, layer))
R_DRAIN_ON_ACT = tuple((p, 0) for p in range(P) if p % 4 != 3)
OUT_DRAIN_ON_ACT = ()
X_BATCHES = [1, 4, 4, 4]
# dummy matmuls to flip the PE HAM to 2.4 GHz; they start right after the
# DVE junk memsets (~4.8us) and should end as the first x batch lands
N_WARMUP_MM = int(os.environ.get("GAT_WARMUP", "6"))

_batch_of = {}
_b0 = 0
for _bi, _bn in enumerate(X_BATCHES):
    for _g in range(_b0, min(_b0 + _bn, P)):
        _batch_of[_g] = _bi
    _b0 += _bn
assert _b0 >= P


def _mm_splits(fd):
    """Split a pair-tick's fd into <=512 matmul chunks."""
    out = []
    j = 0
    while j < fd:
        out.append((j, min(j + MM_N, fd)))
        j += MM_N
    return out


def _build_program() -> bass.Bass:
    nc = bacc.Bacc(None, target_bir_lowering=False, debug=False)

    # xw packs [w0t | xT]: cols 0..95 = W0^T fp16, cols 96.. = x^T shard
    xw = nc.declare_dram_parameter("xw", [D_IN, D_HID + N_PER], F16,
                                   isOutput=False)
    # wb packs [w1t | w2t] fp16
    wb = nc.declare_dram_parameter("wb", [D_HID, D_HID + D_OUT], F16,
                                   isOutput=False)
    yT = nc.declare_dram_parameter("yT", [D_OUT, N_PER], F16, isOutput=True)

    st = {}
    st_batch = {}
    batch_tiles = {}

    with tile.TileContext(nc) as tc:
        with (
            tc.tile_pool(name="consts", bufs=1) as consts,
            tc.tile_pool(name="x0", bufs=1) as x0pool,
            tc.tile_pool(name="xin", bufs=2) as xpool,
            tc.tile_pool(name="sb", bufs=3) as sb,
            tc.tile_pool(name="ps0", bufs=3, space="PSUM") as ps0,
            tc.tile_pool(name="ps1", bufs=3, space="PSUM") as ps1,
            tc.tile_pool(name="ps2", bufs=2, space="PSUM") as ps2,
        ):
            # --- PE warm-up on junk SBUF during the DMA-bound head.
            # memset via DVE: its program loads early and it is otherwise
            # idle; a GpSimd memset would gate the warm-up behind the Pool
            # program load (~6.5us).
            junk_w = consts.tile([D_IN, D_OUT], F16, tag="junkw")
            junk_x = consts.tile([D_IN, MM_N], F16, tag="junkx")
            nc.vector.memset(junk_w[:], 0.0)
            nc.vector.memset(junk_x[:], 0.0)
            # tiny dummy exp: forces the walrus-inserted ACT_TABLE_LOAD
            # (~1.3us) to run during the DMA head instead of gating the
            # first real exp pass.
            junk_e = consts.tile([1, 4], F16, tag="junke")
            nc.scalar.activation(junk_e[:], junk_w[:1, :4], Act.Exp)
            warm = ps2.tile([104, MM_N], F32, tag="p2")
            for _ in range(N_WARMUP_MM):
                nc.tensor.matmul(warm[:D_OUT], junk_w[:], junk_x[:],
                                 start=True, stop=True)

            wb_sb = consts.tile([D_HID, D_HID + D_OUT], F16, tag="wb")
            w1_sb = wb_sb[:, :D_HID]
            w2_sb = wb_sb[:, D_HID:D_HID + D_OUT]

            def relu_drain(out_ap, psum_ap, on_act):
                """out = max(psum, 0), PSUM -> SBUF fp16."""
                if on_act:
                    nc.scalar.activation(out_ap, psum_ap, Act.Relu)
                else:
                    nc.vector.tensor_scalar_max(out_ap, psum_ap, 0.0)

            def exp_elu(p, lyr, psum, fd):
                """From psum: r=max(p,0), t'=min(exp(p),1)-1.

                r + t' = elu(p) exactly, so no +1 inflation or bias
                corrections exist anywhere.  The -1 rides in the t-pass's
                free second ALU op.  PSUM-reading passes stay within one
                512-col bank; the SBUF-side t pass runs full width."""
                e = sb.tile([D_HID, FDP], F16, tag=f"e{lyr}")
                r = sb.tile([D_HID, FDP], F16, tag=f"r{lyr}")
                for j0, j1 in _mm_splits(fd):
                    nc.scalar.activation(e[:, j0:j1], psum[:, j0:j1], Act.Exp)
                    relu_drain(r[:, j0:j1], psum[:, j0:j1],
                               (p, lyr) in R_DRAIN_ON_ACT)
                t = sb.tile([D_HID, FDP], F16, tag=f"t{lyr}")
                nc.vector.tensor_scalar(t[:, :fd], e[:, :fd], 1.0, -1.0,
                                        Alu.min, Alu.add)
                return r, t

            def stage_load(p):
                bi = _batch_of[p]
                if p > 0 and _batch_of[p - 1] == bi:
                    st[p] = st_batch[bi]
                    return
                p1_ = p
                while p1_ + 1 < P and _batch_of[p1_ + 1] == bi:
                    p1_ += 1
                lo = _pstarts[p] + (0 if bi else -D_HID)   # batch 0 incl. w0
                hi = _pstarts[p1_] + _pairs[p1_]
                cols = hi - lo
                pool = x0pool if bi == 0 else xpool
                width = D_HID + FDP * X_BATCHES[0] if bi == 0 else FDP * 4
                xt = pool.tile([D_IN, width], F16,
                               tag=("xt0" if bi == 0 else "xt"))
                nc.sync.dma_start(xt[:, :cols], xw[:, D_HID + lo:D_HID + hi])
                st_batch[bi] = {"xt": xt, "base": lo}
                st[p] = st_batch[bi]

            def stage0_mm(p):
                fd = _pairs[p]
                s = dict(st[p])
                st[p] = s
                xo = _pstarts[p] - s["base"]
                w0_sb = batch_tiles["w0"]
                p0 = ps0.tile([D_HID, FDP], F32, tag="p0")
                for j0, j1 in _mm_splits(fd):
                    nc.tensor.matmul(p0[:, j0:j1], w0_sb,
                                     s["xt"][:, xo + j0:xo + j1],
                                     start=True, stop=True)
                s["p0"] = p0

            def stage0_elu(p):
                s = st[p]
                s["r1"], s["t1"] = exp_elu(p, 0, s.pop("p0"), _pairs[p])

            def stage1_mm(p):
                fd = _pairs[p]
                s = st[p]
                p1 = ps1.tile([D_HID, FDP], F32, tag="p1")
                for j0, j1 in _mm_splits(fd):
                    nc.tensor.matmul(p1[:, j0:j1], w1_sb, s["r1"][:, j0:j1],
                                     start=True, stop=False)
                    nc.tensor.matmul(p1[:, j0:j1], w1_sb, s["t1"][:, j0:j1],
                                     start=False, stop=True)
                s["p1"] = p1

            def stage1_elu(p):
                s = st[p]
                s["r2"], s["t2"] = exp_elu(p, 1, s.pop("p1"), _pairs[p])

            pair_state = {}

            def stage2(p):
                fd = _pairs[p]
                s = st.pop(p)
                if p % 2 == 0:
                    p2 = ps2.tile([104, FDP], F32, tag="p2")
                    pair_state[p // 2] = p2
                    rows = slice(0, D_OUT)
                else:
                    p2 = pair_state[p // 2]
                    rows = slice(64, 64 + D_OUT)
                nc.tensor.matmul(p2[rows, :fd], w2_sb, s["r2"][:, :fd],
                                 start=True, stop=False)
                nc.tensor.matmul(p2[rows, :fd], w2_sb, s["t2"][:, :fd],
                                 start=False, stop=True)
                if not ((p % 2 == 1) or (p == P - 1)):
                    return
                nrows = 104 if p % 2 == 1 else D_OUT
                o = sb.tile([104, FDP], F16, tag="o")
                if p in OUT_DRAIN_ON_ACT:
                    nc.scalar.activation(o[:nrows, :fd], p2[:nrows, :fd],
                                         Act.Identity)
                else:
                    nc.vector.tensor_copy(o[:nrows, :fd], p2[:nrows, :fd])
                # two clean [40, fd] row-slice DMAs per pair into the
                # contiguous yT[40, 6250] (no padded columns)
                g0 = p if p % 2 == 0 else p - 1
                c0 = _pstarts[g0]
                eng = nc.gpsimd if (p // 2) % 2 == 0 else nc.sync
                eng.dma_start(yT[:, c0:c0 + _pairs[g0]],
                              o[:D_OUT, :_pairs[g0]])
                if p % 2 == 1:
                    c1 = _pstarts[p]
                    eng.dma_start(yT[:, c1:c1 + fd],
                                  o[64:64 + D_OUT, :fd])

            # 3-deep software-pipelined emission (measured best: deeper
            # skew spreads PE work thinner, drops the HAM clock gate back
            # to 1.2 GHz and saturates the bufs=3 tile lifetimes).
            for pp in range(P + 3):
                if pp < P:
                    stage_load(pp)
                    if pp == 0:
                        batch_tiles["w0"] = st[0]["xt"][:, 0:D_HID]
                        # consts issue after the first x batch (off the
                        # critical path of the first matmul)
                        nc.sync.dma_start(wb_sb[:], wb[:])
                if 0 <= pp - 1 < P:
                    stage0_mm(pp - 1)
                    stage0_elu(pp - 1)
                if 0 <= pp - 2 < P:
                    stage1_mm(pp - 2)
                    stage1_elu(pp - 2)
                if 0 <= pp - 3 < P:
                    stage2(pp - 3)

    nc.compile()
    return nc


_prog_cache = []
last_result = None


def kernel(**inputs) -> np.ndarray:
    global last_result
    x = np.asarray(inputs["x"], np.float32)           # [50000, 128]
    W0 = np.asarray(inputs["W0"], np.float32).reshape(D_HID, D_IN)
    W1 = np.asarray(inputs["W1"], np.float32).reshape(D_HID, D_HID)
    W2 = np.asarray(inputs["W2"], np.float32).reshape(D_OUT, D_HID)

    n = x.shape[0]
    assert n == N_CORES * N_PER, f"unexpected node count {n}"

    xT16 = x.T.astype(np.float16)                            # [128, 50000]
    w0t = W0.T.astype(np.float16)                            # [128, 96]
    wbm = np.ascontiguousarray(np.concatenate(
        [W1.T.astype(np.float16), W2.T.astype(np.float16)], axis=1))

    if not _prog_cache:
        _prog_cache.append(_build_program())
    nc = _prog_cache[0]

    in_maps = []
    for i in range(N_CORES):
        xwi = np.ascontiguousarray(
            np.concatenate([w0t, xT16[:, i * N_PER:(i + 1) * N_PER]], axis=1))
        in_maps.append(dict(xw=xwi, wb=wbm))
    res = run_bass_kernel_spmd(nc, in_maps, list(range(N_CORES)))
    last_result = res
    out = np.empty((n, D_OUT), np.float32)
    for i in range(N_CORES):
        yt = np.asarray(res.results[i]["yT"], np.float32)  # [40, 6250]
        out[i * N_PER:(i + 1) * N_PER] = yt.T
    return out


if __name__ == "__main__":
    data = np.load("/tmp/gat_inputs.npz")
    y = kernel(**{k: data[k] for k in data.files})
    print("out", y.shape, y.dtype, "absmax", np.abs(y).max())



# revision 27
# speedup vs baseline: 1.3464x; 1.0782x over previous
"""Trainium2 Bass kernel for nn_GAT_87617332838818.

Mathematical collapse: the reference GAT aggregates ``alpha * hp[:, dst]``
over incoming edges per destination node.  Since the softmax weights alpha
sum to exactly 1 within each destination segment and the aggregated message
``hp[dst]`` is constant within the segment, the whole message-passing step
is the identity: ``out[n] = hp[n]``.  The network therefore reduces to a
per-node 3-layer MLP:

    logits = W2r @ elu(W1r @ elu(W0r @ x^T))        (per node column)

with W0r = W0.reshape(96,128), W1r = W1.reshape(96,96), W2r = W2.reshape(40,96)
(head-concat order matches the plain reshape).  Verified numerically against
the reference: rel fro err 4e-7 in f32; 4.5e-3 with this device pipeline.

Device strategy (8 NeuronCores, node-sharded 6250 rows each):
  - activations kept feature-on-partition: xT [128, n], h [96, n]
  - ELU via the split  elu(p') + 1 = max(p',0) + min(exp(p'),1)  with
    p' = p + nb (nb folds the "+1" inflation of the previous layer:
    nb = -W @ ones).  r = max(p+nb,0) and t = min(exp(p+nb),1) are fed
    through TWO accumulating matmuls (linearity), so the inflated h+1 is
    only ever formed in f32 PSUM — bf16-safe.
  - final layer bias cb2 = W2 @ ones subtracted in the output drain pass.
  - pipeline works on 512-column groups (one PSUM bank per matmul).  L2
    outputs of consecutive groups are packed vertically (partitions 0:40
    and 64:104 — PSUM base partitions must be 0/32/64) into one [104,512]
    PSUM tile so one drain pass and paired DMAs cover both groups.
  - PSUM drains split between DVE and ACT for engine balance (any pass
    reading f32 PSUM runs at 1x; only 16-bit SBUF passes get 2x/4x modes).
  - NOTE: engine passes whose PSUM AP spans two banks crashed the device
    (NRT_EXEC_UNIT_UNRECOVERABLE) — keep all PSUM APs within one bank.
  - 3-stage software-pipelined emission so each engine's in-order stream
    always has ready work (avoids head-of-line blocking across pairs).
  - dummy matmuls parked in the DMA-bound head flip the PE HAM clock
    gate to 2.4 GHz before the real matmuls start (measured 427->216 ns).
  - w0 rides in the first x DMA batch; w1/w2 and biases are packed into
    single DMAs to cut ~620 ns/issue sequencer serialization.
"""

import os
import sys

import numpy as np

for _p in ("/root/.axon_site/_ro/trn_rl_repo", "/opt/trn_rl_repo"):
    if os.path.isdir(_p) and _p not in sys.path:
        sys.path.append(_p)

import concourse.bass as bass
import concourse.tile as tile
from concourse import bacc, mybir
from concourse.bass_utils import run_bass_kernel_spmd

N_CORES = 8
N_PER = 6250            # 50000 / 8
D_IN = 128
D_HID = 96
D_OUT = 40
MM_N = 512              # matmul moving free-dim (1 PSUM bank)
FDP = 512               # group free-dim (1 PSUM bank)

F16 = mybir.dt.float16
BF16 = mybir.dt.bfloat16
F32 = mybir.dt.float32

Act = mybir.ActivationFunctionType
Alu = mybir.AluOpType

_pairs = [FDP] * (N_PER // FDP)
if N_PER % FDP:
    _pairs.append(N_PER % FDP)
P = len(_pairs)
_pstarts = [sum(_pairs[:i]) for i in range(P)]

# which L0/L1 relu drains go to ACT instead of DVE (by (pair, layer))
R_DRAIN_ON_ACT = tuple((p, 0) for p in range(P) if p % 4 != 3)
OUT_DRAIN_ON_ACT = ()
X_BATCHES = [1, 4, 4, 4]
# dummy matmuls to flip the PE HAM to 2.4 GHz; they start right after the
# DVE junk memsets (~4.8us) and should end as the first x batch lands
N_WARMUP_MM = int(os.environ.get("GAT_WARMUP", "6"))

_batch_of = {}
_b0 = 0
for _bi, _bn in enumerate(X_BATCHES):
    for _g in range(_b0, min(_b0 + _bn, P)):
        _batch_of[_g] = _bi
    _b0 += _bn
assert _b0 >= P


def _mm_splits(fd):
    """Split a pair-tick's fd into <=512 matmul chunks."""
    out = []
    j = 0
    while j < fd:
        out.append((j, min(j + MM_N, fd)))
        j += MM_N
    return out


def _build_program() -> bass.Bass:
    nc = bacc.Bacc(None, target_bir_lowering=False, debug=False)

    # xw packs [w0t | xT]: cols 0..95 = W0^T fp16, cols 96.. = x^T shard
    xw = nc.declare_dram_parameter("xw", [D_IN, D_HID + N_PER], F16,
                                   isOutput=False)
    # wb packs [w1t | w2t] fp16
    wb = nc.declare_dram_parameter("wb", [D_HID, D_HID + D_OUT], BF16,
                                   isOutput=False)
    yT = nc.declare_dram_parameter("yT", [D_OUT, N_PER], F16, isOutput=True)

    st = {}
    st_batch = {}
    batch_tiles = {}

    with tile.TileContext(nc) as tc:
        with (
            tc.tile_pool(name="consts", bufs=1) as consts,
            tc.tile_pool(name="x0", bufs=1) as x0pool,
            tc.tile_pool(name="xin", bufs=2) as xpool,
            tc.tile_pool(name="sb", bufs=3) as sb,
            tc.tile_pool(name="ps0", bufs=3, space="PSUM") as ps0,
            tc.tile_pool(name="ps1", bufs=3, space="PSUM") as ps1,
            tc.tile_pool(name="ps2", bufs=2, space="PSUM") as ps2,
        ):
            # --- PE warm-up on junk SBUF during the DMA-bound head.
            # memset via DVE: its program loads early and it is otherwise
            # idle; a GpSimd memset would gate the warm-up behind the Pool
            # program load (~6.5us).
            junk_w = consts.tile([D_IN, D_OUT], F16, tag="junkw")
            junk_x = consts.tile([D_IN, MM_N], F16, tag="junkx")
            nc.vector.memset(junk_w[:], 0.0)
            nc.vector.memset(junk_x[:], 0.0)
            # tiny dummy exp: forces the walrus-inserted ACT_TABLE_LOAD
            # (~1.3us) to run during the DMA head instead of gating the
            # first real exp pass.
            junk_e = consts.tile([1, 4], F16, tag="junke")
            nc.scalar.activation(junk_e[:], junk_w[:1, :4], Act.Exp)
            warm = ps2.tile([104, MM_N], F32, tag="p2")
            for _ in range(N_WARMUP_MM):
                nc.tensor.matmul(warm[:D_OUT], junk_w[:], junk_x[:],
                                 start=True, stop=True)

            wb_sb = consts.tile([D_HID, D_HID + D_OUT], BF16, tag="wb")
            w1_sb = wb_sb[:, :D_HID]
            w2_sb = wb_sb[:, D_HID:D_HID + D_OUT]

            def relu_drain(out_ap, psum_ap, on_act):
                """out = max(psum, 0), PSUM -> SBUF fp16."""
                if on_act:
                    nc.scalar.activation(out_ap, psum_ap, Act.Relu)
                else:
                    nc.vector.tensor_scalar_max(out_ap, psum_ap, 0.0)

            def exp_elu(p, lyr, psum, fd):
                """From psum: r=max(p,0), t'=min(exp(p),1)-1.

                r + t' = elu(p) exactly, so no +1 inflation or bias
                corrections exist anywhere.  The -1 rides in the t-pass's
                free second ALU op.  PSUM-reading passes stay within one
                512-col bank; the SBUF-side t pass runs full width."""
                e = sb.tile([D_HID, FDP], BF16, tag=f"e{lyr}")
                r = sb.tile([D_HID, FDP], BF16, tag=f"r{lyr}")
                for j0, j1 in _mm_splits(fd):
                    nc.scalar.activation(e[:, j0:j1], psum[:, j0:j1], Act.Exp)
                    relu_drain(r[:, j0:j1], psum[:, j0:j1],
                               (p, lyr) in R_DRAIN_ON_ACT)
                t = sb.tile([D_HID, FDP], BF16, tag=f"t{lyr}")
                nc.vector.tensor_scalar(t[:, :fd], e[:, :fd], 1.0, -1.0,
                                        Alu.min, Alu.add)
                return r, t

            def stage_load(p):
                bi = _batch_of[p]
                if p > 0 and _batch_of[p - 1] == bi:
                    st[p] = st_batch[bi]
                    return
                p1_ = p
                while p1_ + 1 < P and _batch_of[p1_ + 1] == bi:
                    p1_ += 1
                lo = _pstarts[p] + (0 if bi else -D_HID)   # batch 0 incl. w0
                hi = _pstarts[p1_] + _pairs[p1_]
                cols = hi - lo
                pool = x0pool if bi == 0 else xpool
                width = D_HID + FDP * X_BATCHES[0] if bi == 0 else FDP * 4
                xt = pool.tile([D_IN, width], F16,
                               tag=("xt0" if bi == 0 else "xt"))
                nc.sync.dma_start(xt[:, :cols], xw[:, D_HID + lo:D_HID + hi])
                st_batch[bi] = {"xt": xt, "base": lo}
                st[p] = st_batch[bi]

            def stage0_mm(p):
                fd = _pairs[p]
                s = dict(st[p])
                st[p] = s
                xo = _pstarts[p] - s["base"]
                w0_sb = batch_tiles["w0"]
                p0 = ps0.tile([D_HID, FDP], F32, tag="p0")
                for j0, j1 in _mm_splits(fd):
                    nc.tensor.matmul(p0[:, j0:j1], w0_sb,
                                     s["xt"][:, xo + j0:xo + j1],
                                     start=True, stop=True)
                s["p0"] = p0

            def stage0_elu(p):
                s = st[p]
                s["r1"], s["t1"] = exp_elu(p, 0, s.pop("p0"), _pairs[p])

            def stage1_mm(p):
                fd = _pairs[p]
                s = st[p]
                p1 = ps1.tile([D_HID, FDP], F32, tag="p1")
                for j0, j1 in _mm_splits(fd):
                    nc.tensor.matmul(p1[:, j0:j1], w1_sb, s["r1"][:, j0:j1],
                                     start=True, stop=False)
                    nc.tensor.matmul(p1[:, j0:j1], w1_sb, s["t1"][:, j0:j1],
                                     start=False, stop=True)
                s["p1"] = p1

            def stage1_elu(p):
                s = st[p]
                s["r2"], s["t2"] = exp_elu(p, 1, s.pop("p1"), _pairs[p])

            pair_state = {}

            def stage2(p):
                fd = _pairs[p]
                s = st.pop(p)
                if p % 2 == 0:
                    p2 = ps2.tile([104, FDP], F32, tag="p2")
                    pair_state[p // 2] = p2
                    rows = slice(0, D_OUT)
                else:
                    p2 = pair_state[p // 2]
                    rows = slice(64, 64 + D_OUT)
                nc.tensor.matmul(p2[rows, :fd], w2_sb, s["r2"][:, :fd],
                                 start=True, stop=False)
                nc.tensor.matmul(p2[rows, :fd], w2_sb, s["t2"][:, :fd],
                                 start=False, stop=True)
                if not ((p % 2 == 1) or (p == P - 1)):
                    return
                nrows = 104 if p % 2 == 1 else D_OUT
                o = sb.tile([104, FDP], F16, tag="o")
                if p in OUT_DRAIN_ON_ACT:
                    nc.scalar.activation(o[:nrows, :fd], p2[:nrows, :fd],
                                         Act.Identity)
                else:
                    nc.vector.tensor_copy(o[:nrows, :fd], p2[:nrows, :fd])
                # two clean [40, fd] row-slice DMAs per pair into the
                # contiguous yT[40, 6250] (no padded columns)
                g0 = p if p % 2 == 0 else p - 1
                c0 = _pstarts[g0]
                eng = nc.gpsimd if (p // 2) % 2 == 0 else nc.sync
                eng.dma_start(yT[:, c0:c0 + _pairs[g0]],
                              o[:D_OUT, :_pairs[g0]])
                if p % 2 == 1:
                    c1 = _pstarts[p]
                    eng.dma_start(yT[:, c1:c1 + fd],
                                  o[64:64 + D_OUT, :fd])

            # 3-deep software-pipelined emission (measured best: deeper
            # skew spreads PE work thinner, drops the HAM clock gate back
            # to 1.2 GHz and saturates the bufs=3 tile lifetimes).
            for pp in range(P + 3):
                if pp < P:
                    stage_load(pp)
                    if pp == 0:
                        batch_tiles["w0"] = st[0]["xt"][:, 0:D_HID]
                        # consts issue after the first x batch (off the
                        # critical path of the first matmul)
                        nc.sync.dma_start(wb_sb[:], wb[:])
                if 0 <= pp - 1 < P:
                    stage0_mm(pp - 1)
                    stage0_elu(pp - 1)
                if 0 <= pp - 2 < P:
                    stage1_mm(pp - 2)
                    stage1_elu(pp - 2)
                if 0 <= pp - 3 < P:
                    stage2(pp - 3)

    nc.compile()
    return nc


_prog_cache = []
last_result = None


def kernel(**inputs) -> np.ndarray:
    global last_result
    x = np.asarray(inputs["x"], np.float32)           # [50000, 128]
    W0 = np.asarray(inputs["W0"], np.float32).reshape(D_HID, D_IN)
    W1 = np.asarray(inputs["W1"], np.float32).reshape(D_HID, D_HID)
    W2 = np.asarray(inputs["W2"], np.float32).reshape(D_OUT, D_HID)

    n = x.shape[0]
    assert n == N_CORES * N_PER, f"unexpected node count {n}"

    import ml_dtypes
    xT16 = x.T.astype(np.float16)                            # [128, 50000]
    w0t = W0.T.astype(np.float16)                            # [128, 96]
    wbm = np.ascontiguousarray(np.concatenate(
        [W1.T.astype(ml_dtypes.bfloat16),
         W2.T.astype(ml_dtypes.bfloat16)], axis=1))

    if not _prog_cache:
        _prog_cache.append(_build_program())
    nc = _prog_cache[0]

    in_maps = []
    for i in range(N_CORES):
        xwi = np.ascontiguousarray(
            np.concatenate([w0t, xT16[:, i * N_PER:(i + 1) * N_PER]], axis=1))
        in_maps.append(dict(xw=xwi, wb=wbm))
    res = run_bass_kernel_spmd(nc, in_maps, list(range(N_CORES)))
    last_result = res
    out = np.empty((n, D_OUT), np.float32)
    for i in range(N_CORES):
        yt = np.asarray(res.results[i]["yT"], np.float32)  # [40, 6250]
        out[i * N_PER:(i + 1) * N_PER] = yt.T
    return out


if __name__ == "__main__":
    data = np.load("/tmp/gat_inputs.npz")
    y = kernel(**{k: data[k] for k in data.files})
    print("out", y.shape, y.dtype, "absmax", np.abs(y).max())

